# revision 1
# baseline (speedup 1.0000x reference)
import sys

sys.path.insert(0, '/opt/trn_rl_repo')
import os
import numpy as np

B, T, D = 8192, 4096, 128
NCORES = 8
BC = B // NCORES          # 1024 rows per core
K = 256                   # EMA window (0.9^256 ~ 2e-12 truncation)
NCHUNK = BC // 128        # 8 chunks of 128 rows per core
NUM_ITEMS, NUM_CATS, NUM_STORES, MAX_TIME = 100000, 1000, 10000, 4096
LN09 = float(np.log(0.9))
LN01 = float(np.log(0.1))

_CACHE = {}
LAST_EXEC_NS = None
LAST_TRACE = None


def _build():
    import concourse.bacc as bacc
    import concourse.bass as bass
    import concourse.mybir as mybir
    import concourse.tile as tile

    f32 = mybir.dt.float32
    i32 = mybir.dt.int32
    Alu = mybir.AluOpType
    Act = mybir.ActivationFunctionType

    nc = bacc.Bacc("TRN2", target_bir_lowering=False)

    pop_d = nc.dram_tensor("pop", [BC * T, 1], f32, kind="ExternalInput")
    times_d = nc.dram_tensor("times", [BC], i32, kind="ExternalInput")
    rels_d = nc.dram_tensor("rels", [BC], i32, kind="ExternalInput")
    items_d = nc.dram_tensor("items", [BC], i32, kind="ExternalInput")
    cats_d = nc.dram_tensor("cats", [BC], i32, kind="ExternalInput")
    stores_d = nc.dram_tensor("stores", [BC], i32, kind="ExternalInput")
    rating_d = nc.dram_tensor("rating", [BC], f32, kind="ExternalInput")
    item_emb_d = nc.dram_tensor("item_emb", [NUM_ITEMS + 1, D], f32, kind="ExternalInput")
    time_emb_d = nc.dram_tensor("time_emb", [MAX_TIME + 1, D], f32, kind="ExternalInput")
    cat_emb_d = nc.dram_tensor("cat_emb", [NUM_CATS + 1, D], f32, kind="ExternalInput")
    store_emb_d = nc.dram_tensor("store_emb", [NUM_STORES, D], f32, kind="ExternalInput")
    v5b_d = nc.dram_tensor("v5b", [128, 5 * D], f32, kind="ExternalInput")
    invpow_d = nc.dram_tensor("invpow", [128, K], f32, kind="ExternalInput")
    kidx_d = nc.dram_tensor("kidx", [128, K], f32, kind="ExternalInput")
    scal_d = nc.dram_tensor("scal", [128, 8], f32, kind="ExternalInput")

    ph_o = nc.dram_tensor("ph_o", [BC, 1], f32, kind="ExternalOutput")
    to_o = nc.dram_tensor("to_o", [BC, 1], f32, kind="ExternalOutput")
    so_o = nc.dram_tensor("so_o", [BC, 1], f32, kind="ExternalOutput")
    fin_o = nc.dram_tensor("fin_o", [BC], f32, kind="ExternalOutput")

    IOA = bass.IndirectOffsetOnAxis

    with tile.TileContext(nc) as tc:
        with tc.tile_pool(name="const", bufs=1) as cpool, \
             tc.tile_pool(name="work", bufs=2) as pool:
            v5b = cpool.tile([128, 5 * D], f32)
            invpow = cpool.tile([128, K], f32)
            kidx = cpool.tile([128, K], f32)
            scal = cpool.tile([128, 8], f32)
            rowb = cpool.tile([128, 1], i32)
            cLN01 = cpool.tile([128, 1], f32)

            nc.sync.dma_start(out=v5b[:], in_=v5b_d[:])
            nc.sync.dma_start(out=invpow[:], in_=invpow_d[:])
            nc.sync.dma_start(out=kidx[:], in_=kidx_d[:])
            nc.sync.dma_start(out=scal[:], in_=scal_d[:])
            nc.gpsimd.iota(rowb[:], pattern=[[0, 1]], base=0, channel_multiplier=T)
            nc.vector.memset(cLN01[:], LN01)

            sc_c0 = scal[:, 0:1]
            sc_c1 = scal[:, 1:2]
            sc_bt = scal[:, 2:3]
            sc_w0 = scal[:, 3:4]
            sc_w1 = scal[:, 4:5]
            sc_w2 = scal[:, 5:6]

            for i in range(NCHUNK):
                s, e = i * 128, (i + 1) * 128

                tt = pool.tile([128, 1], i32)
                rl = pool.tile([128, 1], i32)
                it = pool.tile([128, 1], i32)
                ct = pool.tile([128, 1], i32)
                st = pool.tile([128, 1], i32)
                rt = pool.tile([128, 1], f32)
                nc.sync.dma_start(out=tt[:], in_=times_d[s:e, None])
                nc.sync.dma_start(out=rl[:], in_=rels_d[s:e, None])
                nc.sync.dma_start(out=it[:], in_=items_d[s:e, None])
                nc.sync.dma_start(out=ct[:], in_=cats_d[s:e, None])
                nc.sync.dma_start(out=st[:], in_=stores_d[s:e, None])
                nc.sync.dma_start(out=rt[:], in_=rating_d[s:e, None])

                tb = pool.tile([128, 1], i32)
                start = pool.tile([128, 1], i32)
                m_i = pool.tile([128, 1], i32)
                m_f = pool.tile([128, 1], f32)
                pidx0 = pool.tile([128, 1], i32)
                popidx = pool.tile([128, 1], i32)
                nc.vector.tensor_scalar(out=tb[:], in0=tt[:], scalar1=-1, scalar2=0,
                                        op0=Alu.add, op1=Alu.max)
                nc.vector.tensor_scalar(out=start[:], in0=tb[:], scalar1=-(K - 1),
                                        scalar2=0, op0=Alu.add, op1=Alu.max)
                nc.vector.tensor_tensor(out=m_i[:], in0=tb[:], in1=start[:],
                                        op=Alu.subtract)
                nc.vector.tensor_copy(out=m_f[:], in_=m_i[:])
                nc.vector.tensor_tensor(out=pidx0[:], in0=start[:], in1=rowb[:],
                                        op=Alu.add)
                nc.vector.tensor_scalar(out=popidx[:], in0=pidx0[:],
                                        scalar1=s * T, scalar2=None, op0=Alu.add)

                # gathers
                P = pool.tile([128, K], f32)
                G = pool.tile([128, 5 * D], f32)
                nc.gpsimd.indirect_dma_start(
                    out=P[:], out_offset=None, in_=pop_d[:],
                    in_offset=IOA(ap=popidx[:, :1], axis=0))
                for t_i, (tab, idx) in enumerate([
                        (item_emb_d, it), (time_emb_d, tt), (time_emb_d, rl),
                        (cat_emb_d, ct), (store_emb_d, st)]):
                    nc.gpsimd.indirect_dma_start(
                        out=G[:, t_i * D:(t_i + 1) * D], out_offset=None,
                        in_=tab[:], in_offset=IOA(ap=idx[:, :1], axis=0))

                # EMA weights: W[p,k] = 0.1*0.9^(m_p-k) * (k<=m_p), col0 x10 if start==0
                s_p = pool.tile([128, 1], f32)
                W = pool.tile([128, K], f32)
                mask = pool.tile([128, K], f32)
                Wm = pool.tile([128, K], f32)
                e0 = pool.tile([128, 1], f32)
                fac = pool.tile([128, 1], f32)
                nc.scalar.activation(out=s_p[:], in_=m_f[:], func=Act.Exp,
                                     bias=cLN01[:], scale=LN09)
                nc.scalar.activation(out=W[:], in_=invpow[:], func=Act.Copy,
                                     bias=0.0, scale=s_p[:])
                nc.vector.tensor_scalar(out=mask[:], in0=kidx[:], scalar1=m_f[:],
                                        scalar2=None, op0=Alu.is_le)
                nc.vector.tensor_tensor(out=Wm[:], in0=W[:], in1=mask[:], op=Alu.mult)
                nc.vector.tensor_scalar(out=e0[:], in0=start[:], scalar1=0,
                                        scalar2=None, op0=Alu.is_equal)
                nc.vector.tensor_scalar(out=fac[:], in0=e0[:], scalar1=9.0,
                                        scalar2=1.0, op0=Alu.mult, op1=Alu.add)
                nc.vector.tensor_tensor(out=Wm[:, 0:1], in0=Wm[:, 0:1], in1=fac[:],
                                        op=Alu.mult)

                # hist = sum(P*Wm) ; pop_out = sigmoid(hist)
                PW = pool.tile([128, K], f32)
                hist = pool.tile([128, 1], f32)
                ph = pool.tile([128, 1], f32)
                nc.vector.tensor_tensor(out=PW[:], in0=P[:], in1=Wm[:], op=Alu.mult)
                nc.scalar.activation(out=PW[:], in_=PW[:], func=Act.Copy,
                                     bias=0.0, scale=1.0, accum_out=hist[:])
                nc.scalar.activation(out=ph[:], in_=hist[:], func=Act.Sigmoid)

                # embedding dots
                WG = pool.tile([128, 5 * D], f32)
                traw = pool.tile([128, 1], f32)
                cs = pool.tile([128, 1], f32)
                nc.vector.tensor_tensor(out=WG[:], in0=G[:], in1=v5b[:], op=Alu.mult)
                nc.scalar.activation(out=WG[:, 0:3 * D], in_=WG[:, 0:3 * D],
                                     func=Act.Copy, bias=0.0, scale=1.0,
                                     accum_out=traw[:])
                nc.scalar.activation(out=WG[:, 3 * D:5 * D], in_=WG[:, 3 * D:5 * D],
                                     func=Act.Copy, bias=0.0, scale=1.0,
                                     accum_out=cs[:])

                # time_out = sigmoid(leakyrelu(traw + bt))
                tpl = pool.tile([128, 1], f32)
                tps = pool.tile([128, 1], f32)
                tmx = pool.tile([128, 1], f32)
                to = pool.tile([128, 1], f32)
                nc.vector.tensor_scalar(out=tpl[:], in0=traw[:], scalar1=sc_bt,
                                        scalar2=None, op0=Alu.add)
                nc.vector.tensor_scalar(out=tps[:], in0=tpl[:], scalar1=0.01,
                                        scalar2=None, op0=Alu.mult)
                nc.vector.tensor_tensor(out=tmx[:], in0=tpl[:], in1=tps[:], op=Alu.max)
                nc.scalar.activation(out=to[:], in_=tmx[:], func=Act.Sigmoid)

                # side_out = sigmoid(cs + c1*rating + c0)
                s1 = pool.tile([128, 1], f32)
                s2 = pool.tile([128, 1], f32)
                so = pool.tile([128, 1], f32)
                nc.vector.tensor_scalar(out=s1[:], in0=rt[:], scalar1=sc_c1,
                                        scalar2=None, op0=Alu.mult)
                nc.vector.tensor_tensor(out=s2[:], in0=s1[:], in1=cs[:], op=Alu.add)
                nc.scalar.activation(out=so[:], in_=s2[:], func=Act.Sigmoid,
                                     bias=sc_c0, scale=1.0)

                # fin = w0*ph + w1*to + w2*so
                f1 = pool.tile([128, 1], f32)
                f2 = pool.tile([128, 1], f32)
                f3 = pool.tile([128, 1], f32)
                f12 = pool.tile([128, 1], f32)
                fin = pool.tile([128, 1], f32)
                nc.vector.tensor_scalar(out=f1[:], in0=ph[:], scalar1=sc_w0,
                                        scalar2=None, op0=Alu.mult)
                nc.vector.tensor_scalar(out=f2[:], in0=to[:], scalar1=sc_w1,
                                        scalar2=None, op0=Alu.mult)
                nc.vector.tensor_scalar(out=f3[:], in0=so[:], scalar1=sc_w2,
                                        scalar2=None, op0=Alu.mult)
                nc.vector.tensor_tensor(out=f12[:], in0=f1[:], in1=f2[:], op=Alu.add)
                nc.vector.tensor_tensor(out=fin[:], in0=f12[:], in1=f3[:], op=Alu.add)

                nc.sync.dma_start(out=ph_o[s:e, :], in_=ph[:])
                nc.sync.dma_start(out=to_o[s:e, :], in_=to[:])
                nc.sync.dma_start(out=so_o[s:e, :], in_=so[:])
                nc.sync.dma_start(out=fin_o[s:e, None], in_=fin[:])

    nc.compile()
    return nc


def kernel(pop_history, rating_number, item_ids, times, release_times, categories,
           stores, item_emb, cat_emb, store_emb, time_emb,
           W_rating, b_rating, W_side, b_side, W_time, b_time, att_w):
    global LAST_EXEC_NS, LAST_TRACE
    from concourse.bass_utils import run_bass_kernel_spmd

    f32 = np.float32
    pop_history = np.ascontiguousarray(np.asarray(pop_history, f32))
    rating_number = np.asarray(rating_number, f32)
    times_h = np.asarray(times, np.int32)
    rels_h = np.asarray(release_times, np.int32)
    items_h = np.asarray(item_ids, np.int32)
    cats_h = np.asarray(categories, np.int32)
    stores_h = np.asarray(stores, np.int32)
    item_emb = np.ascontiguousarray(np.asarray(item_emb, f32))
    cat_emb = np.ascontiguousarray(np.asarray(cat_emb, f32))
    store_emb = np.ascontiguousarray(np.asarray(store_emb, f32))
    time_emb = np.ascontiguousarray(np.asarray(time_emb, f32))

    f64 = np.float64
    Wt = np.asarray(W_time, f64).reshape(4 * D)
    W1, W2, W3, W4 = Wt[0:D], Wt[D:2 * D], Wt[2 * D:3 * D], Wt[3 * D:4 * D]
    v_item, v_time, v_rel = W2, W3 - W1, W1 + W4
    bt = float(np.asarray(b_time, f64).reshape(-1)[0])

    Ws = np.asarray(W_side, f64).reshape(3 * D)
    Ws1, v_cat, v_store = Ws[0:D], Ws[D:2 * D], Ws[2 * D:3 * D]
    c1 = float(np.asarray(W_rating, f64).reshape(D) @ Ws1)
    c0 = float(np.asarray(b_rating, f64).reshape(D) @ Ws1
               + np.asarray(b_side, f64).reshape(-1)[0])

    aw = np.asarray(att_w, f64).reshape(3)
    ex = np.exp(aw - aw.max())
    sm = ex / ex.sum()

    V5 = np.concatenate([v_item, v_time, v_rel, v_cat, v_store]).astype(f32)
    V5b = np.broadcast_to(V5, (128, 5 * D)).copy()
    kk = np.arange(K, dtype=f64)
    invpow = np.broadcast_to((1.0 / 0.9) ** kk, (128, K)).astype(f32).copy()
    kidx = np.broadcast_to(kk, (128, K)).astype(f32).copy()
    scal = np.zeros((128, 8), f32)
    scal[:, 0] = c0
    scal[:, 1] = c1
    scal[:, 2] = bt
    scal[:, 3] = sm[0]
    scal[:, 4] = sm[1]
    scal[:, 5] = sm[2]

    if "nc" not in _CACHE:
        _CACHE["nc"] = _build()
    nc = _CACHE["nc"]

    in_maps = []
    for c in range(NCORES):
        s, e = c * BC, (c + 1) * BC
        in_maps.append({
            "pop": pop_history[s:e].reshape(-1, 1),
            "times": times_h[s:e], "rels": rels_h[s:e], "items": items_h[s:e],
            "cats": cats_h[s:e], "stores": stores_h[s:e],
            "rating": rating_number[s:e],
            "item_emb": item_emb, "time_emb": time_emb,
            "cat_emb": cat_emb, "store_emb": store_emb,
            "v5b": V5b, "invpow": invpow, "kidx": kidx, "scal": scal,
        })

    trace = bool(os.environ.get("KERNEL_TRACE"))
    res = run_bass_kernel_spmd(nc, in_maps, core_ids=list(range(NCORES)),
                               trace=trace)
    LAST_EXEC_NS = res.exec_time_ns
    LAST_TRACE = res.instructions_and_trace[1] if res.instructions_and_trace else None

    ph = np.concatenate([res.results[c]["ph_o"] for c in range(NCORES)], axis=0)
    to = np.concatenate([res.results[c]["to_o"] for c in range(NCORES)], axis=0)
    so = np.concatenate([res.results[c]["so_o"] for c in range(NCORES)], axis=0)
    fin = np.concatenate([res.results[c]["fin_o"] for c in range(NCORES)], axis=0)
    return (ph.astype(f32), to.astype(f32), so.astype(f32), fin.astype(f32))
